# revision 62
# baseline (speedup 1.0000x reference)
"""Trainium2 Bass kernel for an 8-layer densely-connected MLP (v2).

Math: the reference's dense past/future skip-connection structure is linear
in the per-layer silu outputs a_i, so it collapses (host-side, fp64) to

    a_0 = silu(x @ W0.T + b0)
    a_i = silu(sum_{m<i} a_m @ P[i][m].T + bh[i-1])      i = 1..7
    out = log_softmax(a_7 @ Wout.T + bout)

with 28 precomputed 64x64 matrices P[i][m].

Device layout: activations are feature-major tiles T[i] of [128, chunk]
where each megatile of `mega` batch rows is two chunks A/B of mega/2 rows
living in partition halves.  The half assignment ALTERNATES with layer
parity (even i: A in partitions 0:64; odd i: B in 0:64).  With stationaries
duplicated into both partition halves, every dense term (i, m) maps to two
independent 64x64 PE quadrant matmuls (row group = m%2 side, column group =
i%2 side), so consecutive m terms tile all four PE quadrants with no
activation copies at all.

The schedule software-pipelines ~4 megatiles (round-robin, one layer per
visit, admission paced to the x DMA stream) so the PE engine never drains:
TRN2's PE p-state ramps 0.65->1.2->2.4 GHz only under continuous execution,
so avoiding stalls literally doubles the clock.  Logits are transposed on
the PE in fp16 (fp32 transposes run the array at half rate), PSUM->SBUF
logit staging rides the otherwise-idle gpsimd engine, and log-softmax is
deferred to one batch (a single Exp/Ln table swap) plus a tiny tail segment.
"""

import sys

sys.path.insert(0, "/opt/trn_rl_repo")

import numpy as np

from contextlib import ExitStack

from concourse import bass, mybir, tile
from concourse.bass_utils import run_bass_kernel_spmd

# Problem constants (hardcoded per harness contract)
B, IN, H, OUT, L = 65536, 784, 64, 10, 8
N_CORES = 8
B_CORE = B // N_CORES            # 8192
KBLK = 7                         # K blocks for layer 0
KROW = 112                       # true K-block height (784 = 7*112)

# Megatile sizes: small at the ends (fast pipeline fill, short tail).
MEGA_SCHED = [256, 512, 1024, 1024, 1024, 1024, 1024, 1024, 512, 512, 256]
assert sum(MEGA_SCHED) == B_CORE

f16 = mybir.dt.float16
f32 = mybir.dt.float32
AF = mybir.ActivationFunctionType

W_INFLIGHT = 4                   # megatiles concurrently in the dense pipe
NFILL = 2                        # megatiles in the L0-filler stage
NGEN = W_INFLIGHT + NFILL + 1    # T-tile buffer generations

# emission-time estimates used only to pace megatile admission (build-time)
EST_T0_NS = 6500.0               # preamble before first matmul retires
EST_ARR0_NS = 5500.0             # DMA latency before first x byte lands
EST_BW = 0.345                   # bytes/ns sustained x DMA bandwidth
EST_SLOT_OVH = 55.0              # per-slot fixed overhead (ns)
EST_CYC = 0.42                   # ns per moving column at ramped clock


def make_sched(sizes):
    meta, start, aoff = [], 0, 0
    for mg, mega in enumerate(sizes):
        nblk = mega // 128
        meta.append(
            dict(mg=mg, mega=mega, start=start, chunk=mega // 2, nblk=nblk, aoff=aoff)
        )
        start += mega
        aoff += nblk * OUT
    return meta


SCHED = make_sched(MEGA_SCHED)
ACOLS = sum(m["nblk"] * OUT for m in SCHED)   # 640
NBLK_MAX = max(m["nblk"] for m in SCHED)      # 8

# Pair-packed dense weights: layer i consumes full layer-pairs q < i//2 via
# 128-contraction matmuls on stacked pair tiles, plus (for odd i) one single
# term m = i-1 via a zero-padded full-row stationary.
WIDX = {}                        # (i, q, stream) -> column in wpr
SIDX = {}                        # (i, stream) -> column in wsgl (m = i-1)
for _i in range(2, L):
    for _q in range(_i // 2):
        for _s in range(2):
            WIDX[(_i, _q, _s)] = len(WIDX)
for _i in (1, 3, 5, 7):
    for _s in range(2):
        SIDX[(_i, _s)] = len(SIDX)
NWPR = len(WIDX)                 # 24
NSGL = len(SIDX)                 # 8


def _xoff(sched):
    """Column offset of each megatile slab in the per-core x tensor.
    Slab layout per partition: [ck, j, c] contiguous."""
    offs, off = {}, 0
    for m in sched:
        offs[m["mg"]] = off
        off += 2 * KBLK * m["chunk"]
    return offs, off


XOFFS, XCOLS = _xoff(SCHED)


# ----------------------------------------------------------------------------
# Host-side weight preprocessing
# ----------------------------------------------------------------------------

def _precompute_P(Wh, bh, Wp, Wf):
    """Collapse past/future dense structure into P[(i, m)] (fp64)."""
    Wh = Wh.astype(np.float64)
    Wp = Wp.astype(np.float64)
    Wf = Wf.astype(np.float64)
    nl = L
    Z = np.zeros((H, H))
    S = {}
    for k in range(nl):
        for i in range(nl):
            S[(k, i)] = sum((Wf[k * (nl - 1) + (j - 1)] for j in range(i + 1, nl)), start=Z)
    G = {(0, 0): np.eye(H)}
    for i in range(1, nl):
        G[(i, i)] = np.eye(H) + S[(i, i)] if i < nl - 1 else np.eye(H)
        for m in range(i):
            G[(i, m)] = sum((S[(k, i)] @ G[(k, m)] for k in range(m, i)), start=Z)
    P = {}
    for i in range(1, nl):
        C = {j: Wh[i - 1] @ Wp[j * (nl - 1) + (i - 1)] for j in range(i)}
        for m in range(i):
            P[(i, m)] = sum((C[j] @ G[(j, m)] for j in range(m, i)), start=Z)
    return P


def _pack_weights(W0, b0, Wh, bh, Wp, Wf, Wout, bout):
    P = _precompute_P(Wh, bh, Wp, Wf)
    # W0.T in K-blocks of 112 true rows (no padding): [112, 7, 64]
    w0t = np.ascontiguousarray(
        W0.astype(np.float64).T.reshape(KBLK, KROW, H).transpose(1, 0, 2)
    ).astype(np.float16)
    # Pair stationaries: stream A stacks (P[i][2q] ; P[i][2q+1]) to match the
    # MA pair tile [a_2q(A) ; a_2q+1(A)]; stream B the reverse for MB.
    wpr = np.zeros((128, NWPR, H), np.float16)
    for (i, q, s), k in WIDX.items():
        lo = P[(i, 2 * q)].T.astype(np.float16)
        hi = P[(i, 2 * q + 1)].T.astype(np.float16)
        wpr[0:H, k] = lo if s == 0 else hi
        wpr[H:128, k] = hi if s == 0 else lo
    # Single-term stationaries (m = i-1 even, A-half in T[m] rows 0:64):
    # full 128-row with the other stream's half zeroed.
    wsgl = np.zeros((128, NSGL, H), np.float16)
    for (i, s), k in SIDX.items():
        pt = P[(i, i - 1)].T.astype(np.float16)
        rb = 0 if s == 0 else H
        wsgl[rb : rb + H, k] = pt
    # Output stationaries zero-padded per stream: s=0 pairs with T7's A-half
    # (rows 64:128, layer 7 odd), s=1 with the B-half (rows 0:64).
    woutt_d = np.zeros((128, 2, OUT), np.float16)
    woutt_d[H:128, 0] = Wout.T.astype(np.float16)
    woutt_d[0:H, 1] = Wout.T.astype(np.float16)
    # per-layer biases duplicated into both halves; cols L..2L-1 hold the
    # negated biases for the exp-table silu path: [128, 16]
    bias8 = np.zeros((128, 2 * L), np.float32)
    bias8[0:H, 0] = b0
    bias8[H:128, 0] = b0
    for i in range(1, L):
        bias8[0:H, i] = bh[i - 1]
        bias8[H:128, i] = bh[i - 1]
    bias8[:, L : 2 * L] = -bias8[:, 0:L]
    boutb = np.tile(bout.astype(np.float32), (128, NBLK_MAX))
    ident = np.eye(OUT, dtype=np.float16)
    ident32 = np.eye(OUT, dtype=np.float32)
    return dict(
        w0t=w0t, wpr=wpr, wsgl=wsgl, woutt_d=woutt_d, bias8=bias8, boutb=boutb,
        ident=ident, ident32=ident32,
    )


# ----------------------------------------------------------------------------
# Device program
# ----------------------------------------------------------------------------

def build_nc(sched=None, silu_via_sigmoid=False, upfront_dma=True):
    nc = bass.Bass()
    sched = SCHED if sched is None else sched
    nmt = len(sched)
    acols = sum(m["nblk"] * OUT for m in sched)
    xoffs, xcols = _xoff(sched)

    xt_e = nc.dram_tensor("xt", [KROW, xcols], f16, kind="ExternalInput")
    w0t_e = nc.dram_tensor("w0t", [KROW, KBLK, H], f16, kind="ExternalInput")
    wpr_e = nc.dram_tensor("wpr", [128, NWPR, H], f16, kind="ExternalInput")
    wsgl_e = nc.dram_tensor("wsgl", [128, NSGL, H], f16, kind="ExternalInput")
    woutt_e = nc.dram_tensor("woutt_d", [128, 2, OUT], f16, kind="ExternalInput")
    bias8_e = nc.dram_tensor("bias8", [128, 2 * L], f32, kind="ExternalInput")
    boutb_e = nc.dram_tensor("boutb", [128, OUT * NBLK_MAX], f32, kind="ExternalInput")
    ident_e = nc.dram_tensor("ident", [OUT, OUT], f16, kind="ExternalInput")
    ident32_e = nc.dram_tensor("ident32", [OUT, OUT], f32, kind="ExternalInput")
    o_e = nc.dram_tensor("o", [128, acols], f32, kind="ExternalOutput")

    with tile.TileContext(nc) as tc, ExitStack() as ctx:
        consts = ctx.enter_context(tc.tile_pool(name="consts", bufs=1))
        xpool = ctx.enter_context(tc.tile_pool(name="xpool", bufs=1))
        tpool = ctx.enter_context(tc.tile_pool(name="tpool", bufs=1))

        apool = ctx.enter_context(tc.tile_pool(name="apool", bufs=1))
        pp = ctx.enter_context(tc.tile_pool(name="pp", bufs=5, space="PSUM"))
        pl0 = ctx.enter_context(tc.tile_pool(name="pl0", bufs=2, space="PSUM"))
        p2 = ctx.enter_context(tc.tile_pool(name="p2", bufs=1, space="PSUM"))

        # --- DMA triggers: x stream on the SP ring, consts on the ACT ring
        xts = {}

        XBUFS = 6

        def trigger_x(m):
            mg, chunk = m["mg"], m["chunk"]
            xc = xpool.tile(
                [KROW, 2, KBLK, 512], f16, tag=f"x{mg % XBUFS}", name=f"x{mg}"
            )[:, :, :, :chunk]
            off = xoffs[mg]
            nc.sync.dma_start(
                xc[:],
                xt_e[:, off : off + 2 * KBLK * chunk].rearrange(
                    "p (ck j c) -> p ck j c", ck=2, j=KBLK
                ),
            )
            xts[mg] = xc

        w0t_s = consts.tile([KROW, KBLK, H], f16)
        wpr_s = consts.tile([128, NWPR, H], f16)
        wsgl_s = consts.tile([128, NSGL, H], f16)
        woutt_s = consts.tile([128, 2, OUT], f16)
        bias_s = consts.tile([128, 2 * L], f32)
        boutb_s = consts.tile([128, OUT * NBLK_MAX], f32)

        # everything rides the SP ring: first slab + critical weights first,
        # then the x stream (a separate consts ring gets starved by the flood)
        trigger_x(sched[0])
        nc.sync.dma_start(bias_s[:], bias8_e[:])
        nc.sync.dma_start(w0t_s[:], w0t_e[:])
        nc.sync.dma_start(wsgl_s[:], wsgl_e[:])
        trigger_x(sched[1])
        nc.sync.dma_start(wpr_s[:], wpr_e[:])
        trigger_x(sched[2])
        nc.sync.dma_start(woutt_s[:], woutt_e[:])
        nc.sync.dma_start(boutb_s[:], boutb_e[:])
        for m in sched[3 : (len(sched) if upfront_dma else W_INFLIGHT)]:
            trigger_x(m)

        # Prime ACT (loads the Silu table during the DMA prologue) and DVE.
        prim_a = consts.tile([128, 1], f32)
        prime_fn = AF.Sigmoid if silu_via_sigmoid else AF.Silu
        nc.scalar.activation(prim_a[:], bias_s[:, 0:1], prime_fn)
        prim_v = consts.tile([128, 1], f32)
        nc.vector.tensor_copy(prim_v[:], bias_s[:, 0:1])

        out_acc = apool.tile([128, acols], f32)
        ex = apool.tile([128, acols], f32)
        sm = apool.tile([128, acols // OUT], f32)
        lsm = apool.tile([128, acols // OUT], f32)
        od = apool.tile([128, acols], f32)

        # ---------------- emission-time pacing estimate -------------------
        est = dict(ns=EST_T0_NS)
        arrivals = []
        cum = 0.7e6  # consts precede/interleave the x stream on the SP ring
        for m in sched:
            cum += m["mega"] * IN * 2
            arrivals.append(EST_ARR0_NS + cum / EST_BW)

        def est_add_slot(cols):
            est["ns"] += cols * EST_CYC + EST_SLOT_OVH

        # ---------------- per-megatile emission helpers -------------------
        Ts = {}
        Ps = {}

        def alloc_T(m):
            mg = m["mg"]
            Ts[mg] = [
                tpool.tile(
                    [128, 512], f16, tag=f"T{i}_{mg % NGEN}", name=f"T{i}_{mg}"
                )[:, : m["chunk"]]
                for i in range(L)
            ]
            Ps[mg] = {}

        def emit_silu(m, i, ps, use_exp=False):
            dst = Ts[m["mg"]][i][:]
            if use_exp:
                # silu via the Exp table: z/(1+exp(-z)) — lets ACT swap to the
                # Exp/Ln table before the final megatile so the batch softmax
                # overlaps the pipeline drain
                ex8 = tpool.tile(
                    [128, 512], f32, tag="esg", name="esg", bufs=2
                )[:, : m["chunk"]]
                nc.scalar.activation(
                    ex8[:], ps[:], AF.Exp,
                    bias=bias_s[:, L + i : L + i + 1], scale=-1.0,
                )
                nc.vector.tensor_scalar_add(ex8[:], ex8[:], 1.0)
                gg = tpool.tile(
                    [128, 512], f32, tag="gsg", name="gsg", bufs=2
                )[:, : m["chunk"]]
                nc.vector.reciprocal(gg[:], ex8[:])
                nc.vector.scalar_tensor_tensor(
                    out=dst, in0=ps[:], scalar=bias_s[:, i : i + 1], in1=gg[:],
                    op0=mybir.AluOpType.add, op1=mybir.AluOpType.mult,
                )
            elif not silu_via_sigmoid:
                nc.scalar.activation(dst, ps[:], AF.Silu, bias=bias_s[:, i : i + 1])
            else:  # CoreSim lacks Silu; mathematically identical path
                sg = tpool.tile(
                    [128, 512], f32, tag="sg", name="sg", bufs=2
                )[:, : m["chunk"]]
                nc.scalar.activation(
                    sg[:], ps[:], AF.Sigmoid, bias=bias_s[:, i : i + 1]
                )
                nc.vector.scalar_tensor_tensor(
                    out=dst, in0=ps[:], scalar=bias_s[:, i : i + 1], in1=sg[:],
                    op0=mybir.AluOpType.add, op1=mybir.AluOpType.mult,
                )
            est["ns"] += 150.0

        def emit_l0_slot(m, ps, j):
            mg, chunk = m["mg"], m["chunk"]
            xc = xts[mg]
            first = j == 0
            last = j == KBLK - 1
            nc.tensor.matmul(
                ps[0:H, :], w0t_s[:, j, :], xc[:, 0, j, :],
                start=first, stop=last, skip_group_check=True,
            )
            nc.tensor.matmul(
                ps[H:128, :], w0t_s[:, j, :], xc[:, 1, j, :],
                start=first, stop=last, skip_group_check=True,
            )
            est_add_slot(chunk)

        def emit_pair_copies(m, q):
            # MA = [a_2q(A) ; a_2q+1(A)], MB = [a_2q+1(B) ; a_2q(B)] — the
            # parity layout makes all four copies partition-shift-free.
            mg, chunk = m["mg"], m["chunk"]
            T = Ts[mg]
            ma = tpool.tile(
                [128, 512], f16, tag=f"MA{q}_{mg % NGEN}", name=f"MA{q}_{mg}"
            )[:, :chunk]
            mb = tpool.tile(
                [128, 512], f16, tag=f"MB{q}_{mg % NGEN}", name=f"MB{q}_{mg}"
            )[:, :chunk]
            nc.vector.tensor_copy(ma[0:H, :], T[2 * q][0:H, :])
            nc.vector.tensor_copy(ma[H:128, :], T[2 * q + 1][H:128, :])
            nc.vector.tensor_copy(mb[0:H, :], T[2 * q + 1][0:H, :])
            nc.vector.tensor_copy(mb[H:128, :], T[2 * q][H:128, :])
            Ps[mg][q] = (ma, mb)

        def emit_dense(m, i):
            mg, chunk = m["mg"], m["chunk"]
            T = Ts[mg]
            pa = 64 * (i % 2)      # A-half output partitions for this layer
            pb = 64 - pa
            nq = i // 2
            nslots = nq + (i % 2)
            ps = pp.tile([128, 512], f32, tag="ps", name=f"ps{i}_{mg}")[:, :chunk]
            for q in range(nq):
                first = q == 0
                last = q == nslots - 1
                ma, mb = Ps[mg][q]
                nc.tensor.matmul(
                    ps[pa : pa + 64, :], wpr_s[:, WIDX[(i, q, 0)], :], ma[:, :],
                    start=first, stop=last, skip_group_check=True,
                )
                nc.tensor.matmul(
                    ps[pb : pb + 64, :], wpr_s[:, WIDX[(i, q, 1)], :], mb[:, :],
                    start=first, stop=last, skip_group_check=True,
                )
                est_add_slot(chunk)
            if i % 2:
                first = nq == 0
                nc.tensor.matmul(
                    ps[pa : pa + 64, :], wsgl_s[:, SIDX[(i, 0)], :], T[i - 1][:, :],
                    start=first, stop=True, skip_group_check=True,
                )
                nc.tensor.matmul(
                    ps[pb : pb + 64, :], wsgl_s[:, SIDX[(i, 1)], :], T[i - 1][:, :],
                    start=first, stop=True, skip_group_check=True,
                )
                est_add_slot(chunk)
            emit_silu(m, i, ps, use_exp=use_exp_silu(m["mg"]))
            if i % 2 == 1 and i < L - 1:
                emit_pair_copies(m, i // 2)

        def emit_out(m):
            # batch-major logits directly: per 128-batch block, the T7 slab is
            # the STATIONARY and Wout.T the (10-col) moving tensor, so
            # out = T7_blk.T @ Wout.T lands pre-transposed in PSUM.
            mg, chunk, nblk = m["mg"], m["chunk"], m["nblk"]
            T7 = Ts[mg][L - 1]  # odd layer: B in 0:64, A in 64:128
            pt = p2.tile([128, OUT * NBLK_MAX], f32, tag="pt", name=f"pt{mg}")
            nba = nblk // 2
            for blk in range(nblk):
                s = 0 if blk < nba else 1           # A blocks first, then B
                cs = (blk % nba) * 128
                nc.tensor.matmul(
                    pt[:, blk * OUT : (blk + 1) * OUT],
                    T7[:, cs : cs + 128],
                    woutt_s[:, s, :],
                    start=True, stop=True, skip_group_check=True,
                )
            est["ns"] += 150.0 * nblk
            nc.vector.tensor_add(
                out_acc[:, m["aoff"] : m["aoff"] + OUT * nblk],
                pt[:, : OUT * nblk],
                boutb_s[:, 0 : OUT * nblk],
            )

        def emit_softmax(c0, c1, last):
            g0, g1 = c0 // OUT, c1 // OUT
            nc.scalar.activation(ex[:, c0:c1], out_acc[:, c0:c1], AF.Exp)
            nc.vector.reduce_sum(
                out=sm[:, g0:g1],
                in_=ex[:, c0:c1].rearrange("p (g c) -> p g c", c=OUT),
                axis=mybir.AxisListType.X,
            )
            nc.scalar.activation(lsm[:, g0:g1], sm[:, g0:g1], AF.Ln)
            nc.vector.tensor_sub(
                od[:, c0:c1].rearrange("p (g c) -> p g c", c=OUT),
                out_acc[:, c0:c1].rearrange("p (g c) -> p g c", c=OUT),
                lsm[:, g0:g1].unsqueeze(2).broadcast_to([128, g1 - g0, OUT]),
            )
            nc.sync.dma_start(o_e[:, c0:c1], od[:, c0:c1])

        # ---------------- software-pipelined schedule ---------------------
        # Future megatiles' L0 K-block slots are emitted as FILLER between
        # dense stages: they depend only on the x DMA, so they plug every
        # silu-latency bubble and keep the PE fed.
        state = dict(main_left=(nmt - 1) * L, ret_main=0, sm_done=False)
        inflight = []
        filling = []                 # [mg, next_j, ps_tile]
        pending = list(range(nmt))
        layer_of = {}

        def use_exp_silu(mg):
            # once every other megatile's silus are emitted, the final
            # megatile's remaining silus ride the Exp table so ACT swaps
            # tables exactly once, well before the tail
            return nmt > 1 and mg == nmt - 1 and state["main_left"] == 0

        def note_silu(mg):
            if mg != nmt - 1:
                state["main_left"] -= 1

        def admit():
            mg = pending.pop(0)
            m = sched[mg]
            if not upfront_dma and mg + W_INFLIGHT < len(sched):
                trigger_x(sched[mg + W_INFLIGHT])
            alloc_T(m)
            ps = pl0.tile([128, 512], f32, tag="psl0", name=f"ps0_{mg}")
            filling.append([mg, 0, ps[:, : m["chunk"]]])

        def emit_filler(nslots):
            while nslots > 0 and filling:
                ent = filling[0]
                mg, j, ps = ent
                emit_l0_slot(sched[mg], ps, j)
                nslots -= 1
                if j == KBLK - 1:
                    note_silu(mg)
                    emit_silu(sched[mg], 0, ps, use_exp=use_exp_silu(mg))
                    layer_of[mg] = 1
                    inflight.append(mg)
                    filling.pop(0)
                else:
                    ent[1] += 1

        def can_admit():
            if not pending or len(filling) >= NFILL:
                return False
            if len(inflight) + len(filling) >= W_INFLIGHT + NFILL:
                return False
            if len(inflight) + len(filling) < 2:
                return True
            return est["ns"] >= arrivals[pending[0]] - 2200.0

        while inflight or filling or pending:
            while can_admit():
                admit()
            if not inflight:
                emit_filler(KBLK)
                continue
            mg = inflight.pop(0)
            m = sched[mg]
            i = layer_of[mg]
            note_silu(mg)
            emit_dense(m, i)
            if i == L - 1:
                emit_out(m)
                if mg != nmt - 1:
                    state["ret_main"] += 1
                elif state["sm_done"]:
                    emit_softmax(sched[-1]["aoff"], acols, last=True)
                else:
                    emit_softmax(0, acols, last=True)
                    state["sm_done"] = True
            else:
                layer_of[mg] = i + 1
                inflight.append(mg)
            if (
                not state["sm_done"]
                and state["main_left"] == 0
                and state["ret_main"] == nmt - 1
                and nmt > 1
            ):
                # every other megatile's out_acc rows are final: batch-softmax
                # them under the final megatile's remaining work
                emit_softmax(0, sched[-1]["aoff"], last=False)
                state["sm_done"] = True
            emit_filler(2)

    _split_multi_waits(nc)
    return nc


def _split_multi_waits(nc):
    """walrus's activation encoding admits one sync-wait; hoist extras onto
    preceding same-engine NoOps (sequentially equivalent)."""
    for blk in nc.m.functions[0].blocks:
        idx = 0
        while idx < len(blk.instructions):
            inst = blk.instructions[idx]
            si = inst.sync_info
            splittable = isinstance(
                inst,
                (
                    mybir.InstActivation,
                    mybir.InstTensorCopy,
                    mybir.InstTensorTensor,
                    mybir.InstTensorReduce,
                    mybir.InstTensorScalarPtr,
                    mybir.InstReciprocal,
                    mybir.InstMatmult,
                    mybir.InstLdweights,
                    mybir.InstDMACopy,
                    mybir.InstMemset,
                    mybir.InstDrain,
                    mybir.InstStreamTranspose,
                ),
            )
            if splittable and si is not None and len(si.on_wait) > 1:
                extras = list(si.on_wait[:-1])
                si.on_wait = [si.on_wait[-1]]
                for w in reversed(extras):
                    nop = mybir.InstNoOp(
                        name=nc.get_next_instruction_name(), ins=[], outs=[]
                    )
                    nop.engine = inst.engine
                    nop.sync_info = mybir.SyncInfo(on_wait=[w], on_update=[])
                    nc.register_instruction(nop)
                    blk.instructions.insert(idx, nop)
                    idx += 1
            idx += 1


# ----------------------------------------------------------------------------
# Host wrapper
# ----------------------------------------------------------------------------

_CACHE = {}


def _get_nc():
    if "nc" not in _CACHE:
        _CACHE["nc"] = build_nc()
    return _CACHE["nc"]


def pack_x(x_slice, sched=None):
    """[rows, 784] fp32 -> per-core tiled layout [128, XCOLS] fp16: one slab
    per megatile, per-partition [ck, j, c] contiguous."""
    sched = SCHED if sched is None else sched
    xoffs, xcols = _xoff(sched)
    xt16 = x_slice.T.astype(np.float16).reshape(KBLK, KROW, -1)
    out = np.empty((KROW, xcols), np.float16)
    for m in sched:
        chunk = m["chunk"]
        off = xoffs[m["mg"]]
        blk = xt16[:, :, m["start"] : m["start"] + m["mega"]]
        # [j, p, 2*chunk] -> [p, ck, j, c]
        blk = blk.reshape(KBLK, KROW, 2, chunk).transpose(1, 2, 0, 3)
        out[:, off : off + 2 * KBLK * chunk] = blk.reshape(KROW, -1)
    return out


def prepare_inputs(x, W0, b0, Wh, bh, Wp, Wf, Wout, bout):
    consts = _pack_weights(W0, b0, Wh, bh, Wp, Wf, Wout, bout)
    in_maps = []
    for c in range(N_CORES):
        m = dict(consts)
        m["xt"] = pack_x(x[c * B_CORE : (c + 1) * B_CORE])
        in_maps.append(m)
    return in_maps


def _unpermute(o_core, sched=None):
    sched = SCHED if sched is None else sched
    b_core = sum(m["mega"] for m in sched)
    out = np.empty((b_core, OUT), np.float32)
    for m in sched:
        seg = o_core[:, m["aoff"] : m["aoff"] + m["nblk"] * OUT]
        seg = seg.reshape(128, m["nblk"], OUT).transpose(1, 0, 2)
        out[m["start"] : m["start"] + m["mega"]] = seg.reshape(m["mega"], OUT)
    return out


def run(inputs, trace=False, **kw):
    in_maps = prepare_inputs(**inputs)
    nc = _get_nc()
    res = run_bass_kernel_spmd(nc, in_maps, list(range(N_CORES)), trace=trace, **kw)
    out = np.empty((B, OUT), np.float32)
    for c in range(N_CORES):
        out[c * B_CORE : (c + 1) * B_CORE] = _unpermute(res.results[c]["o"])
    return out, res


def kernel(**inputs):
    out, _ = run(inputs, trace=False)
    return out


# revision 65
# speedup vs baseline: 1.1156x; 1.1156x over previous
"""Trainium2 Bass kernel for an 8-layer densely-connected MLP (v2).

Math: the reference's dense past/future skip-connection structure is linear
in the per-layer silu outputs a_i, so it collapses (host-side, fp64) to

    a_0 = silu(x @ W0.T + b0)
    a_i = silu(sum_{m<i} a_m @ P[i][m].T + bh[i-1])      i = 1..7
    out = log_softmax(a_7 @ Wout.T + bout)

with 28 precomputed 64x64 matrices P[i][m].

Device layout: activations are feature-major tiles T[i] of [128, chunk]
where each megatile of `mega` batch rows is two chunks A/B of mega/2 rows
living in partition halves.  The half assignment ALTERNATES with layer
parity (even i: A in partitions 0:64; odd i: B in 0:64).  With stationaries
duplicated into both partition halves, every dense term (i, m) maps to two
independent 64x64 PE quadrant matmuls (row group = m%2 side, column group =
i%2 side), so consecutive m terms tile all four PE quadrants with no
activation copies at all.

The schedule software-pipelines ~4 megatiles (round-robin, one layer per
visit, admission paced to the x DMA stream) so the PE engine never drains:
TRN2's PE p-state ramps 0.65->1.2->2.4 GHz only under continuous execution,
so avoiding stalls literally doubles the clock.  Logits are transposed on
the PE in fp16 (fp32 transposes run the array at half rate), PSUM->SBUF
logit staging rides the otherwise-idle gpsimd engine, and log-softmax is
deferred to one batch (a single Exp/Ln table swap) plus a tiny tail segment.
"""

import sys

sys.path.insert(0, "/opt/trn_rl_repo")

import numpy as np

from contextlib import ExitStack

from concourse import bass, mybir, tile
from concourse.bass_utils import run_bass_kernel_spmd

# Problem constants (hardcoded per harness contract)
B, IN, H, OUT, L = 65536, 784, 64, 10, 8
N_CORES = 8
B_CORE = B // N_CORES            # 8192
KBLK = 7                         # K blocks for layer 0
KROW = 112                       # true K-block height (784 = 7*112)

# Megatile sizes: small at the ends (fast pipeline fill, short tail).
MEGA_SCHED = [512, 1024, 1024, 1024, 1024, 1024, 1024, 1024, 512]
assert sum(MEGA_SCHED) == B_CORE

f16 = mybir.dt.float16
f32 = mybir.dt.float32
AF = mybir.ActivationFunctionType

W_INFLIGHT = 4                   # megatiles concurrently in the dense pipe
NFILL = 2                        # megatiles in the L0-filler stage
NGEN = W_INFLIGHT + NFILL + 1    # T-tile buffer generations

# emission-time estimates used only to pace megatile admission (build-time)
EST_T0_NS = 6500.0               # preamble before first matmul retires
EST_ARR0_NS = 5500.0             # DMA latency before first x byte lands
EST_BW = 0.345                   # bytes/ns sustained x DMA bandwidth
EST_SLOT_OVH = 55.0              # per-slot fixed overhead (ns)
EST_CYC = 0.42                   # ns per moving column at ramped clock


def make_sched(sizes):
    meta, start, aoff = [], 0, 0
    for mg, mega in enumerate(sizes):
        nblk = mega // 128
        meta.append(
            dict(mg=mg, mega=mega, start=start, chunk=mega // 2, nblk=nblk, aoff=aoff)
        )
        start += mega
        aoff += nblk * OUT
    return meta


SCHED = make_sched(MEGA_SCHED)
ACOLS = sum(m["nblk"] * OUT for m in SCHED)   # 640
NBLK_MAX = max(m["nblk"] for m in SCHED)      # 8

# Pair-packed dense weights: layer i consumes full layer-pairs q < i//2 via
# 128-contraction matmuls on stacked pair tiles, plus (for odd i) one single
# term m = i-1 via a zero-padded full-row stationary.
WIDX = {}                        # (i, q, stream) -> column in wpr
SIDX = {}                        # (i, stream) -> column in wsgl (m = i-1)
for _i in range(2, L):
    for _q in range(_i // 2):
        for _s in range(2):
            WIDX[(_i, _q, _s)] = len(WIDX)
for _i in (1, 3, 5, 7):
    for _s in range(2):
        SIDX[(_i, _s)] = len(SIDX)
NWPR = len(WIDX)                 # 24
NSGL = len(SIDX)                 # 8


def _xoff(sched):
    """Column offset of each megatile slab in the per-core x tensor.
    Slab layout per partition: [ck, j, c] contiguous."""
    offs, off = {}, 0
    for m in sched:
        offs[m["mg"]] = off
        off += 2 * KBLK * m["chunk"]
    return offs, off


XOFFS, XCOLS = _xoff(SCHED)


# ----------------------------------------------------------------------------
# Host-side weight preprocessing
# ----------------------------------------------------------------------------

def _precompute_P(Wh, bh, Wp, Wf):
    """Collapse past/future dense structure into P[(i, m)] (fp64)."""
    Wh = Wh.astype(np.float64)
    Wp = Wp.astype(np.float64)
    Wf = Wf.astype(np.float64)
    nl = L
    Z = np.zeros((H, H))
    S = {}
    for k in range(nl):
        for i in range(nl):
            S[(k, i)] = sum((Wf[k * (nl - 1) + (j - 1)] for j in range(i + 1, nl)), start=Z)
    G = {(0, 0): np.eye(H)}
    for i in range(1, nl):
        G[(i, i)] = np.eye(H) + S[(i, i)] if i < nl - 1 else np.eye(H)
        for m in range(i):
            G[(i, m)] = sum((S[(k, i)] @ G[(k, m)] for k in range(m, i)), start=Z)
    P = {}
    for i in range(1, nl):
        C = {j: Wh[i - 1] @ Wp[j * (nl - 1) + (i - 1)] for j in range(i)}
        for m in range(i):
            P[(i, m)] = sum((C[j] @ G[(j, m)] for j in range(m, i)), start=Z)
    return P


def _pack_weights(W0, b0, Wh, bh, Wp, Wf, Wout, bout):
    P = _precompute_P(Wh, bh, Wp, Wf)
    # W0.T in K-blocks of 112 true rows (no padding): [112, 7, 64]
    w0t = np.ascontiguousarray(
        W0.astype(np.float64).T.reshape(KBLK, KROW, H).transpose(1, 0, 2)
    ).astype(np.float16)
    # Pair stationaries: stream A stacks (P[i][2q] ; P[i][2q+1]) to match the
    # MA pair tile [a_2q(A) ; a_2q+1(A)]; stream B the reverse for MB.
    wpr = np.zeros((128, NWPR, H), np.float16)
    for (i, q, s), k in WIDX.items():
        lo = P[(i, 2 * q)].T.astype(np.float16)
        hi = P[(i, 2 * q + 1)].T.astype(np.float16)
        wpr[0:H, k] = lo if s == 0 else hi
        wpr[H:128, k] = hi if s == 0 else lo
    # Single-term stationaries (m = i-1 even, A-half in T[m] rows 0:64):
    # full 128-row with the other stream's half zeroed.
    wsgl = np.zeros((128, NSGL, H), np.float16)
    for (i, s), k in SIDX.items():
        pt = P[(i, i - 1)].T.astype(np.float16)
        rb = 0 if s == 0 else H
        wsgl[rb : rb + H, k] = pt
    # Output stationaries zero-padded per stream: s=0 pairs with T7's A-half
    # (rows 64:128, layer 7 odd), s=1 with the B-half (rows 0:64).
    woutt_d = np.zeros((128, 2, OUT), np.float16)
    woutt_d[H:128, 0] = Wout.T.astype(np.float16)
    woutt_d[0:H, 1] = Wout.T.astype(np.float16)
    # per-layer biases duplicated into both halves; cols L..2L-1 hold the
    # negated biases for the exp-table silu path: [128, 16]
    bias8 = np.zeros((128, 2 * L), np.float32)
    bias8[0:H, 0] = b0
    bias8[H:128, 0] = b0
    for i in range(1, L):
        bias8[0:H, i] = bh[i - 1]
        bias8[H:128, i] = bh[i - 1]
    bias8[:, L : 2 * L] = -bias8[:, 0:L]
    boutb = np.tile(bout.astype(np.float32), (128, NBLK_MAX))
    ident = np.eye(OUT, dtype=np.float16)
    ident32 = np.eye(OUT, dtype=np.float32)
    return dict(
        w0t=w0t, wpr=wpr, wsgl=wsgl, woutt_d=woutt_d, bias8=bias8, boutb=boutb,
        ident=ident, ident32=ident32,
    )


# ----------------------------------------------------------------------------
# Device program
# ----------------------------------------------------------------------------

def build_nc(sched=None, silu_via_sigmoid=False, upfront_dma=True):
    nc = bass.Bass()
    sched = SCHED if sched is None else sched
    nmt = len(sched)
    acols = sum(m["nblk"] * OUT for m in sched)
    xoffs, xcols = _xoff(sched)

    xt_e = nc.dram_tensor("xt", [KROW, xcols], f16, kind="ExternalInput")
    w0t_e = nc.dram_tensor("w0t", [KROW, KBLK, H], f16, kind="ExternalInput")
    wpr_e = nc.dram_tensor("wpr", [128, NWPR, H], f16, kind="ExternalInput")
    wsgl_e = nc.dram_tensor("wsgl", [128, NSGL, H], f16, kind="ExternalInput")
    woutt_e = nc.dram_tensor("woutt_d", [128, 2, OUT], f16, kind="ExternalInput")
    bias8_e = nc.dram_tensor("bias8", [128, 2 * L], f32, kind="ExternalInput")
    boutb_e = nc.dram_tensor("boutb", [128, OUT * NBLK_MAX], f32, kind="ExternalInput")
    ident_e = nc.dram_tensor("ident", [OUT, OUT], f16, kind="ExternalInput")
    ident32_e = nc.dram_tensor("ident32", [OUT, OUT], f32, kind="ExternalInput")
    o_e = nc.dram_tensor("o", [128, acols], f32, kind="ExternalOutput")

    with tile.TileContext(nc) as tc, ExitStack() as ctx:
        consts = ctx.enter_context(tc.tile_pool(name="consts", bufs=1))
        xpool = ctx.enter_context(tc.tile_pool(name="xpool", bufs=1))
        tpool = ctx.enter_context(tc.tile_pool(name="tpool", bufs=1))

        apool = ctx.enter_context(tc.tile_pool(name="apool", bufs=1))
        pp = ctx.enter_context(tc.tile_pool(name="pp", bufs=5, space="PSUM"))
        pl0 = ctx.enter_context(tc.tile_pool(name="pl0", bufs=2, space="PSUM"))
        p2 = ctx.enter_context(tc.tile_pool(name="p2", bufs=1, space="PSUM"))

        # --- DMA triggers: x stream on the SP ring, consts on the ACT ring
        xts = {}

        XBUFS = 5

        def trigger_x(m):
            mg, chunk = m["mg"], m["chunk"]
            xc = xpool.tile(
                [KROW, 2, KBLK, 512], f16, tag=f"x{mg % XBUFS}", name=f"x{mg}"
            )[:, :, :, :chunk]
            off = xoffs[mg]
            nc.sync.dma_start(
                xc[:],
                xt_e[:, off : off + 2 * KBLK * chunk].rearrange(
                    "p (ck j c) -> p ck j c", ck=2, j=KBLK
                ),
            )
            xts[mg] = xc

        w0t_s = consts.tile([KROW, KBLK, H], f16)
        wpr_s = consts.tile([128, NWPR, H], f16)
        wsgl_s = consts.tile([128, NSGL, H], f16)
        woutt_s = consts.tile([128, 2, OUT], f16)
        bias_s = consts.tile([128, 2 * L], f32)
        boutb_s = consts.tile([128, OUT * NBLK_MAX], f32)

        # everything rides the SP ring: first slab + critical weights first,
        # then the x stream (a separate consts ring gets starved by the flood)
        trigger_x(sched[0])
        nc.sync.dma_start(bias_s[:], bias8_e[:])
        nc.sync.dma_start(w0t_s[:], w0t_e[:])
        nc.sync.dma_start(wsgl_s[:], wsgl_e[:])
        trigger_x(sched[1])
        nc.sync.dma_start(wpr_s[:], wpr_e[:])
        trigger_x(sched[2])
        nc.sync.dma_start(woutt_s[:], woutt_e[:])
        nc.sync.dma_start(boutb_s[:], boutb_e[:])
        for m in sched[3 : (len(sched) if upfront_dma else W_INFLIGHT)]:
            trigger_x(m)

        # Prime ACT (loads the Silu table during the DMA prologue) and DVE.
        prim_a = consts.tile([128, 1], f32)
        prime_fn = AF.Sigmoid if silu_via_sigmoid else AF.Silu
        nc.scalar.activation(prim_a[:], bias_s[:, 0:1], prime_fn)
        prim_v = consts.tile([128, 1], f32)
        nc.vector.tensor_copy(prim_v[:], bias_s[:, 0:1])

        out_acc = apool.tile([128, acols], f32)
        ex = apool.tile([128, acols], f32)
        sm = apool.tile([128, acols // OUT], f32)
        lsm = apool.tile([128, acols // OUT], f32)
        od = apool.tile([128, acols], f32)

        # ---------------- emission-time pacing estimate -------------------
        est = dict(ns=EST_T0_NS)
        arrivals = []
        cum = 0.7e6  # consts precede/interleave the x stream on the SP ring
        for m in sched:
            cum += m["mega"] * IN * 2
            arrivals.append(EST_ARR0_NS + cum / EST_BW)

        def est_add_slot(cols):
            est["ns"] += cols * EST_CYC + EST_SLOT_OVH

        # ---------------- per-megatile emission helpers -------------------
        Ts = {}
        Ps = {}

        def alloc_T(m):
            mg = m["mg"]
            Ts[mg] = [
                tpool.tile(
                    [128, 512], f16, tag=f"T{i}_{mg % NGEN}", name=f"T{i}_{mg}"
                )[:, : m["chunk"]]
                for i in range(L)
            ]
            Ps[mg] = {}

        def emit_silu(m, i, ps, use_exp=False):
            dst = Ts[m["mg"]][i][:]
            if use_exp:
                # silu via the Exp table: z/(1+exp(-z)) — lets ACT swap to the
                # Exp/Ln table before the final megatile so the batch softmax
                # overlaps the pipeline drain
                ex8 = tpool.tile(
                    [128, 512], f32, tag="esg", name="esg", bufs=2
                )[:, : m["chunk"]]
                nc.scalar.activation(
                    ex8[:], ps[:], AF.Exp,
                    bias=bias_s[:, L + i : L + i + 1], scale=-1.0,
                )
                nc.vector.tensor_scalar_add(ex8[:], ex8[:], 1.0)
                gg = tpool.tile(
                    [128, 512], f32, tag="gsg", name="gsg", bufs=2
                )[:, : m["chunk"]]
                nc.vector.reciprocal(gg[:], ex8[:])
                nc.vector.scalar_tensor_tensor(
                    out=dst, in0=ps[:], scalar=bias_s[:, i : i + 1], in1=gg[:],
                    op0=mybir.AluOpType.add, op1=mybir.AluOpType.mult,
                )
            elif not silu_via_sigmoid:
                nc.scalar.activation(dst, ps[:], AF.Silu, bias=bias_s[:, i : i + 1])
            else:  # CoreSim lacks Silu; mathematically identical path
                sg = tpool.tile(
                    [128, 512], f32, tag="sg", name="sg", bufs=2
                )[:, : m["chunk"]]
                nc.scalar.activation(
                    sg[:], ps[:], AF.Sigmoid, bias=bias_s[:, i : i + 1]
                )
                nc.vector.scalar_tensor_tensor(
                    out=dst, in0=ps[:], scalar=bias_s[:, i : i + 1], in1=sg[:],
                    op0=mybir.AluOpType.add, op1=mybir.AluOpType.mult,
                )
            est["ns"] += 150.0

        def emit_l0_slot(m, ps, j):
            mg, chunk = m["mg"], m["chunk"]
            xc = xts[mg]
            first = j == 0
            last = j == KBLK - 1
            nc.tensor.matmul(
                ps[0:H, :], w0t_s[:, j, :], xc[:, 0, j, :],
                start=first, stop=last, skip_group_check=True,
            )
            nc.tensor.matmul(
                ps[H:128, :], w0t_s[:, j, :], xc[:, 1, j, :],
                start=first, stop=last, skip_group_check=True,
            )
            est_add_slot(chunk)

        def emit_pair_copies(m, q):
            # MA = [a_2q(A) ; a_2q+1(A)], MB = [a_2q+1(B) ; a_2q(B)] — the
            # parity layout makes all four copies partition-shift-free.
            mg, chunk = m["mg"], m["chunk"]
            T = Ts[mg]
            ma = tpool.tile(
                [128, 512], f16, tag=f"MA{q}_{mg % NGEN}", name=f"MA{q}_{mg}"
            )[:, :chunk]
            mb = tpool.tile(
                [128, 512], f16, tag=f"MB{q}_{mg % NGEN}", name=f"MB{q}_{mg}"
            )[:, :chunk]
            nc.vector.tensor_copy(ma[0:H, :], T[2 * q][0:H, :])
            nc.vector.tensor_copy(ma[H:128, :], T[2 * q + 1][H:128, :])
            nc.vector.tensor_copy(mb[0:H, :], T[2 * q + 1][0:H, :])
            nc.vector.tensor_copy(mb[H:128, :], T[2 * q][H:128, :])
            Ps[mg][q] = (ma, mb)

        def emit_dense(m, i):
            mg, chunk = m["mg"], m["chunk"]
            T = Ts[mg]
            pa = 64 * (i % 2)      # A-half output partitions for this layer
            pb = 64 - pa
            nq = i // 2
            nslots = nq + (i % 2)
            ps = pp.tile([128, 512], f32, tag="ps", name=f"ps{i}_{mg}")[:, :chunk]
            for q in range(nq):
                first = q == 0
                last = q == nslots - 1
                ma, mb = Ps[mg][q]
                nc.tensor.matmul(
                    ps[pa : pa + 64, :], wpr_s[:, WIDX[(i, q, 0)], :], ma[:, :],
                    start=first, stop=last, skip_group_check=True,
                )
                nc.tensor.matmul(
                    ps[pb : pb + 64, :], wpr_s[:, WIDX[(i, q, 1)], :], mb[:, :],
                    start=first, stop=last, skip_group_check=True,
                )
                est_add_slot(chunk)
            if i % 2:
                first = nq == 0
                nc.tensor.matmul(
                    ps[pa : pa + 64, :], wsgl_s[:, SIDX[(i, 0)], :], T[i - 1][:, :],
                    start=first, stop=True, skip_group_check=True,
                )
                nc.tensor.matmul(
                    ps[pb : pb + 64, :], wsgl_s[:, SIDX[(i, 1)], :], T[i - 1][:, :],
                    start=first, stop=True, skip_group_check=True,
                )
                est_add_slot(chunk)
            emit_silu(m, i, ps, use_exp=use_exp_silu(m["mg"]))
            if i % 2 == 1 and i < L - 1:
                emit_pair_copies(m, i // 2)

        def emit_out(m):
            # batch-major logits directly: per 128-batch block, the T7 slab is
            # the STATIONARY and Wout.T the (10-col) moving tensor, so
            # out = T7_blk.T @ Wout.T lands pre-transposed in PSUM.
            mg, chunk, nblk = m["mg"], m["chunk"], m["nblk"]
            T7 = Ts[mg][L - 1]  # odd layer: B in 0:64, A in 64:128
            pt = p2.tile([128, OUT * NBLK_MAX], f32, tag="pt", name=f"pt{mg}")
            nba = nblk // 2
            for blk in range(nblk):
                s = 0 if blk < nba else 1           # A blocks first, then B
                cs = (blk % nba) * 128
                nc.tensor.matmul(
                    pt[:, blk * OUT : (blk + 1) * OUT],
                    T7[:, cs : cs + 128],
                    woutt_s[:, s, :],
                    start=True, stop=True, skip_group_check=True,
                )
            est["ns"] += 150.0 * nblk
            nc.vector.tensor_add(
                out_acc[:, m["aoff"] : m["aoff"] + OUT * nblk],
                pt[:, : OUT * nblk],
                boutb_s[:, 0 : OUT * nblk],
            )

        def emit_softmax(c0, c1, last):
            g0, g1 = c0 // OUT, c1 // OUT
            nc.scalar.activation(ex[:, c0:c1], out_acc[:, c0:c1], AF.Exp)
            nc.vector.reduce_sum(
                out=sm[:, g0:g1],
                in_=ex[:, c0:c1].rearrange("p (g c) -> p g c", c=OUT),
                axis=mybir.AxisListType.X,
            )
            nc.scalar.activation(lsm[:, g0:g1], sm[:, g0:g1], AF.Ln)
            nc.vector.tensor_sub(
                od[:, c0:c1].rearrange("p (g c) -> p g c", c=OUT),
                out_acc[:, c0:c1].rearrange("p (g c) -> p g c", c=OUT),
                lsm[:, g0:g1].unsqueeze(2).broadcast_to([128, g1 - g0, OUT]),
            )
            nc.sync.dma_start(o_e[:, c0:c1], od[:, c0:c1])

        # ---------------- software-pipelined schedule ---------------------
        # Future megatiles' L0 K-block slots are emitted as FILLER between
        # dense stages: they depend only on the x DMA, so they plug every
        # silu-latency bubble and keep the PE fed.
        state = dict(main_left=(nmt - 1) * L, ret_main=0, sm_done=False)
        inflight = []
        filling = []                 # [mg, next_j, ps_tile]
        pending = list(range(nmt))
        layer_of = {}

        def use_exp_silu(mg):
            # (disabled: the 3 extra DVE ops per layer lengthen the
            # latency-critical drain chain more than the saved table load)
            return False

        def note_silu(mg):
            if mg != nmt - 1:
                state["main_left"] -= 1

        def admit():
            mg = pending.pop(0)
            m = sched[mg]
            if not upfront_dma and mg + W_INFLIGHT < len(sched):
                trigger_x(sched[mg + W_INFLIGHT])
            alloc_T(m)
            ps = pl0.tile([128, 512], f32, tag="psl0", name=f"ps0_{mg}")
            filling.append([mg, 0, ps[:, : m["chunk"]]])

        def emit_filler(nslots):
            while nslots > 0 and filling:
                ent = filling[0]
                mg, j, ps = ent
                emit_l0_slot(sched[mg], ps, j)
                nslots -= 1
                if j == KBLK - 1:
                    note_silu(mg)
                    emit_silu(sched[mg], 0, ps, use_exp=use_exp_silu(mg))
                    layer_of[mg] = 1
                    inflight.append(mg)
                    filling.pop(0)
                else:
                    ent[1] += 1

        def can_admit():
            if not pending or len(filling) >= NFILL:
                return False
            if len(inflight) + len(filling) >= W_INFLIGHT + NFILL:
                return False
            if len(inflight) + len(filling) < 2:
                return True
            return est["ns"] >= arrivals[pending[0]] - 2200.0

        while inflight or filling or pending:
            while can_admit():
                admit()
            if not inflight:
                emit_filler(KBLK)
                continue
            mg = inflight.pop(0)
            m = sched[mg]
            i = layer_of[mg]
            note_silu(mg)
            emit_dense(m, i)
            if i == L - 1:
                emit_out(m)
                if mg != nmt - 1:
                    state["ret_main"] += 1
                elif state["sm_done"]:
                    emit_softmax(sched[-1]["aoff"], acols, last=True)
                else:
                    emit_softmax(0, acols, last=True)
                    state["sm_done"] = True
            else:
                layer_of[mg] = i + 1
                inflight.append(mg)
            if (
                not state["sm_done"]
                and state["main_left"] == 0
                and state["ret_main"] == nmt - 1
                and nmt > 1
            ):
                # every other megatile's out_acc rows are final: batch-softmax
                # them under the final megatile's remaining work
                emit_softmax(0, sched[-1]["aoff"], last=False)
                state["sm_done"] = True
            emit_filler(2)

    _split_multi_waits(nc)
    return nc


def _split_multi_waits(nc):
    """walrus's activation encoding admits one sync-wait; hoist extras onto
    preceding same-engine NoOps (sequentially equivalent)."""
    for blk in nc.m.functions[0].blocks:
        idx = 0
        while idx < len(blk.instructions):
            inst = blk.instructions[idx]
            si = inst.sync_info
            splittable = isinstance(
                inst,
                (
                    mybir.InstActivation,
                    mybir.InstTensorCopy,
                    mybir.InstTensorTensor,
                    mybir.InstTensorReduce,
                    mybir.InstTensorScalarPtr,
                    mybir.InstReciprocal,
                    mybir.InstMatmult,
                    mybir.InstLdweights,
                    mybir.InstDMACopy,
                    mybir.InstMemset,
                    mybir.InstDrain,
                    mybir.InstStreamTranspose,
                ),
            )
            if splittable and si is not None and len(si.on_wait) > 1:
                extras = list(si.on_wait[:-1])
                si.on_wait = [si.on_wait[-1]]
                for w in reversed(extras):
                    nop = mybir.InstNoOp(
                        name=nc.get_next_instruction_name(), ins=[], outs=[]
                    )
                    nop.engine = inst.engine
                    nop.sync_info = mybir.SyncInfo(on_wait=[w], on_update=[])
                    nc.register_instruction(nop)
                    blk.instructions.insert(idx, nop)
                    idx += 1
            idx += 1


# ----------------------------------------------------------------------------
# Host wrapper
# ----------------------------------------------------------------------------

_CACHE = {}


def _get_nc():
    if "nc" not in _CACHE:
        _CACHE["nc"] = build_nc()
    return _CACHE["nc"]


def pack_x(x_slice, sched=None):
    """[rows, 784] fp32 -> per-core tiled layout [128, XCOLS] fp16: one slab
    per megatile, per-partition [ck, j, c] contiguous."""
    sched = SCHED if sched is None else sched
    xoffs, xcols = _xoff(sched)
    xt16 = x_slice.T.astype(np.float16).reshape(KBLK, KROW, -1)
    out = np.empty((KROW, xcols), np.float16)
    for m in sched:
        chunk = m["chunk"]
        off = xoffs[m["mg"]]
        blk = xt16[:, :, m["start"] : m["start"] + m["mega"]]
        # [j, p, 2*chunk] -> [p, ck, j, c]
        blk = blk.reshape(KBLK, KROW, 2, chunk).transpose(1, 2, 0, 3)
        out[:, off : off + 2 * KBLK * chunk] = blk.reshape(KROW, -1)
    return out


def prepare_inputs(x, W0, b0, Wh, bh, Wp, Wf, Wout, bout):
    consts = _pack_weights(W0, b0, Wh, bh, Wp, Wf, Wout, bout)
    in_maps = []
    for c in range(N_CORES):
        m = dict(consts)
        m["xt"] = pack_x(x[c * B_CORE : (c + 1) * B_CORE])
        in_maps.append(m)
    return in_maps


def _unpermute(o_core, sched=None):
    sched = SCHED if sched is None else sched
    b_core = sum(m["mega"] for m in sched)
    out = np.empty((b_core, OUT), np.float32)
    for m in sched:
        seg = o_core[:, m["aoff"] : m["aoff"] + m["nblk"] * OUT]
        seg = seg.reshape(128, m["nblk"], OUT).transpose(1, 0, 2)
        out[m["start"] : m["start"] + m["mega"]] = seg.reshape(m["mega"], OUT)
    return out


def run(inputs, trace=False, **kw):
    in_maps = prepare_inputs(**inputs)
    nc = _get_nc()
    res = run_bass_kernel_spmd(nc, in_maps, list(range(N_CORES)), trace=trace, **kw)
    out = np.empty((B, OUT), np.float32)
    for c in range(N_CORES):
        out[c * B_CORE : (c + 1) * B_CORE] = _unpermute(res.results[c]["o"])
    return out, res


def kernel(**inputs):
    out, _ = run(inputs, trace=False)
    return out
